# revision 1
# baseline (speedup 1.0000x reference)
"""AutoInt (nn_AutoInt_51101520888215) distributed Trainium2 kernel.

Strategy (per sharding hint): pure data-parallel over the batch across the
8 NeuronCores. The 1M x 16 embedding table and the small Q/K/V/res/output
weights are replicated to every core; each core gathers its own 1024x39
embedding rows locally (no collectives needed) and computes the full
AutoInt forward for its batch shard. Host only slices the batch, folds
Wq/Wk into per-head bilinear matrices (A_h = Wq_h @ Wk_h^T, a weight-only
preprocessing), and concatenates the 8 per-core [1024, 1] outputs.

B, F, D, P, H = 8192, 39, 16, 16, 8 are hardcoded per the problem spec.
"""

import numpy as np

B, F, D, P, H, V = 8192, 39, 16, 16, 8, 1000000
NCORES = 8
BS = B // NCORES  # 1024 samples per core

_COMPILED = {}


def _device_fn():
    """Build the 8-way SPMD (pmap) function (cached; one compile)."""
    if "fn" in _COMPILED:
        return _COMPILED["fn"]
    import jax
    import jax.numpy as jnp

    def fwd(idx, table, acat, wv, wres, out_w, out_b):
        # idx: [BS, F] int32; table: [V, D] f32
        e = table[idx]  # [BS, F, D] gather on device
        # scores_h = e @ A_h @ e^T  (A_h = Wq_h Wk_h^T folded on host)
        t = jnp.einsum("bfd,dhp->bhfp", e, acat)        # [BS,H,F,P]
        s = jnp.einsum("bhqp,bkp->bhqk", t, e)          # [BS,H,F,F]
        # softmax over the QUERY axis (dim=2) - per reference
        s = s - jnp.max(s, axis=2, keepdims=True)
        es = jnp.exp(s)
        att = es / jnp.sum(es, axis=2, keepdims=True)
        v = jnp.einsum("bfd,dhp->bhfp", e, wv)          # [BS,H,F,P]
        av = jnp.einsum("bhqk,bhkp->bhqp", att, v)      # [BS,H,F,P]
        mh = jnp.transpose(av, (0, 2, 1, 3)).reshape(BS, F, H * P)
        mh = mh + jnp.einsum("bfd,dk->bfk", e, wres)
        mh = jax.nn.relu(mh).reshape(BS, F * H * P)
        y = jax.nn.sigmoid(mh @ out_w + out_b)          # [BS,1]
        return y

    _COMPILED["fn"] = jax.pmap(fwd, devices=jax.devices()[:NCORES])
    return _COMPILED["fn"]


def kernel(feat_index, emb_table, Wq, Wk, Wv, Wres, out_W, out_b):
    import jax
    import jax.numpy as jnp

    feat_index = np.asarray(feat_index)
    emb_table = np.asarray(emb_table, dtype=np.float32)
    Wq = np.asarray(Wq, dtype=np.float32)
    Wk = np.asarray(Wk, dtype=np.float32)
    Wv = np.asarray(Wv, dtype=np.float32)
    Wres = np.asarray(Wres, dtype=np.float32)
    out_W = np.asarray(out_W, dtype=np.float32)
    out_b = np.asarray(out_b, dtype=np.float32)

    # ---- host-side weight folding (O(D^2 H P), tiny) ----
    # A_h = Wq_h @ Wk_h^T  -> scores = e A_h e^T per head.
    Wq_h = Wq.reshape(D, H, P).transpose(1, 0, 2)   # [H, D, P]
    Wk_h = Wk.reshape(D, H, P).transpose(1, 0, 2)   # [H, D, P]
    A = np.einsum("hdp,hep->hde", Wq_h, Wk_h)       # [H, D, D]
    acat = A.transpose(1, 0, 2)                     # [D, H, Dk] -> e@A: bfd,dhp
    wv_r = Wv.reshape(D, H, P)                      # [D, H, P]

    idx32 = feat_index.astype(np.int32)             # values < 1M fit in int32

    fn = _device_fn()

    # shard the batch [8, BS, F]; replicate table + weights on every core
    rep = lambda a: np.broadcast_to(a, (NCORES,) + a.shape)
    out = fn(
        idx32.reshape(NCORES, BS, F),
        rep(emb_table),
        rep(acat.astype(np.float32)),
        rep(wv_r),
        rep(Wres),
        rep(out_W),
        rep(out_b),
    )
    # gather/unshard
    return np.asarray(out).reshape(B, 1).astype(np.float32)

